# revision 15
# baseline (speedup 1.0000x reference)
"""Trainium2 Bass kernel for nn_Backbone_20332375179599.

For binary position tensors top/left [B, L, N] and an indicator [B, L]:

    D[b,i,j] = sum_n |top[b,i,n]-top[b,j,n]| + sum_n |left[b,i,n]-left[b,j,n]|
    out = (D * (1-cls_m) + 100*pad_m) * (1 + 100*(1 - sep_i*sep_j))

Because the positions are binary, |a-b| = a(1-2b) + b, so with
u = [top; left] (K = 576 rows, one column per token) and w = 1-2u:

    D[i,j] = sum_k u[k,i]*w[k,j] + sum_k 1*u[k,j]

i.e. D lands directly in PSUM from two matmul sets over the same K
chunks (lhsT = query columns of u / all-ones). The masks are rank-2 and
rank-4 outer products of per-token indicator vectors, each one tiny
matmul into its own PSUM bank:

    out = D * B2 + B3
    B2[i,j] = r_i r_j (101 - 100 q_i q_j)          (r = not-CLS, q = SEP)
    B3[i,j] = (100 - 100 p_i p_j)(101 - 100 q_i q_j)   (p = not-PAD)

All values are small integers: bf16 operands / fp32 accumulate are exact.

Sharding: 8 cores = 2 batches x 4 query-row blocks of 128. Each core
computes a [128, 512] slab of the [B, 512, 512] output.
"""

import sys

sys.path.insert(0, "/opt/trn_rl_repo")

import numpy as np
import ml_dtypes

B, L, NNODE = 2, 512, 288
KTOT = 2 * NNODE  # 576
KPAD = 640  # padded to 5 chunks of 128
NCHUNK = 5
N_CORES = 8
CORES_PER_BATCH = 4
QROWS = L // CORES_PER_BATCH  # 128

_CACHE = {}


def _build_module():
    import concourse.mybir as mybir
    import concourse.tile as tile
    from concourse import bacc

    f32 = mybir.dt.float32
    bf16 = mybir.dt.bfloat16
    fp8 = mybir.dt.float8e4

    nc = bacc.Bacc(
        "TRN2", target_bir_lowering=False, debug=False, num_devices=N_CORES
    )

    # Columns are key-rotated per core so this core's 128 query tokens sit
    # at columns 0:128 (the host un-rotates the output). u and w are
    # separate DMAs so the u-side matmuls start before w lands.
    u_d = nc.dram_tensor("u", [KPAD, L], fp8, kind="ExternalInput").ap()
    w_d = nc.dram_tensor("w", [KPAD, L], fp8, kind="ExternalInput").ap()
    # m rows 0:2 = B2 operands, rows 32:36 = B3 operands (matmul base
    # partition must be 0/32/64); cols [rhs L | lhsT QROWS]
    m_d = nc.dram_tensor("m", [36, L + QROWS], bf16, kind="ExternalInput").ap()
    out_d = nc.dram_tensor("out", [QROWS, L], f32, kind="ExternalOutput").ap()

    with tile.TileContext(nc) as tc:
        _kernel_body(tc, mybir, out_d, u_d, w_d, m_d)

    nc.compile()
    return nc


def _kernel_body(tc, mybir, out_d, u_d, w_d, m_d):
    nc = tc.nc
    f32 = mybir.dt.float32
    bf16 = mybir.dt.bfloat16
    fp8 = mybir.dt.float8e4
    DR = mybir.MatmulPerfMode.DoubleRow
    Alu = mybir.AluOpType
    UW = L + QROWS  # 640

    with (
        tc.tile_pool(name="sb", bufs=1) as sb,
        tc.tile_pool(name="ps", bufs=1, space="PSUM") as ps,
    ):
        # One DMA per DRAM tensor, tiny m first, then u, then w (HWDGE and
        # the DMA engines serialize; u gates more matmuls than w). u/w land
        # chunk-major: chunk c is u_sb[:, c, :], full-K row index = c*128+p.
        u_sb = sb.tile([128, NCHUNK, L], fp8, tag="u")
        w_sb = sb.tile([128, NCHUNK, L], fp8, tag="w")
        m_sb = sb.tile([36, UW], bf16, tag="m")
        nc.sync.dma_start(m_sb[:, :], m_d[:, :])
        nc.sync.dma_start(
            u_sb[:, :, :], u_d.rearrange("(c p) n -> p c n", p=128)
        )
        nc.sync.dma_start(
            w_sb[:, :, :], w_d.rearrange("(c p) n -> p c n", p=128)
        )

        ones_sb = sb.tile([128, 2, QROWS], fp8, tag="ones")
        nc.vector.memset(ones_sb[:, :, :], 1.0)

        psum_d = ps.tile([QROWS, L], f32, tag="psum_d")
        psum_b2 = ps.tile([QROWS, L], f32, tag="psum_b2")
        psum_b3 = ps.tile([QROWS, L], f32, tag="psum_b3")

        # mask banks: tiny matmuls, data ready early, warm up PE
        nc.tensor.matmul(
            psum_b2[:, :], m_sb[0:2, L:UW], m_sb[0:2, :L], start=True, stop=True
        )
        nc.tensor.matmul(
            psum_b3[:, :], m_sb[32:36, L:UW], m_sb[32:36, :L], start=True, stop=True
        )

        # B2 to SBUF early (off the critical path) so the epilogue
        # ops each have at most one PSUM operand.
        b2_sb = sb.tile([QROWS, L], f32, tag="b2_sb")
        nc.scalar.copy(b2_sb[:, :], psum_b2[:, :])

        # D = sum_c [ ones.T @ u_c  +  uq_c.T @ w_c ], fp8 DoubleRow on
        # chunk pairs (0,1) and (2,3), plain fp8 on chunk 4. The query
        # block is columns 0:QROWS of the rotated u.
        nc.tensor.matmul(
            psum_d[:, :], ones_sb[:, :, :], u_sb[:, 0:2, :],
            start=True, stop=False, perf_mode=DR,
        )
        nc.tensor.matmul(
            psum_d[:, :], ones_sb[:, :, :], u_sb[:, 2:4, :],
            start=False, stop=False, perf_mode=DR,
        )
        nc.tensor.matmul(
            psum_d[:, :], ones_sb[:, 0, :], u_sb[:, 4, :],
            start=False, stop=False,
        )
        nc.tensor.matmul(
            psum_d[:, :], u_sb[:, 0:2, :QROWS], w_sb[:, 0:2, :],
            start=False, stop=False, perf_mode=DR,
        )
        nc.tensor.matmul(
            psum_d[:, :], u_sb[:, 2:4, :QROWS], w_sb[:, 2:4, :],
            start=False, stop=False, perf_mode=DR,
        )
        nc.tensor.matmul(
            psum_d[:, :], u_sb[:, 4, :QROWS], w_sb[:, 4, :],
            start=False, stop=True,
        )

        t_sb = sb.tile([QROWS, L], f32, tag="t_sb")
        nc.vector.tensor_tensor(
            out=t_sb[:, :], in0=psum_d[:, :], in1=b2_sb[:, :], op=Alu.mult
        )
        # o = 100 * B3' + t   (B3 was shipped scaled by 1/100 to stay
        # bf16-exact: values {±1, ±100, ±101})
        o_sb = sb.tile([QROWS, L], f32, tag="o_sb")
        nc.vector.scalar_tensor_tensor(
            out=o_sb[:, :], in0=psum_b3[:, :], scalar=100.0, in1=t_sb[:, :],
            op0=Alu.mult, op1=Alu.add,
        )

        nc.sync.dma_start(out_d[:, :], o_sb[:, :])


def _get_nc():
    if "nc" not in _CACHE:
        _CACHE["nc"] = _build_module()
    return _CACHE["nc"]


def _pack_m(m2, m3):
    m = np.zeros((36, m2.shape[1]), m2.dtype)
    m[0:2] = m2
    m[32:36] = m3
    return m


def _make_in_maps(entire_top, entire_left, indicator):
    bf16 = ml_dtypes.bfloat16
    fp8 = ml_dtypes.float8_e4m3
    in_maps = []
    per_batch = {}
    for b in range(B):
        u = np.zeros((KPAD, L), np.float32)
        u[:KTOT] = np.concatenate([entire_top[b], entire_left[b]], axis=1).T
        w = np.zeros((KPAD, L), np.float32)
        w[:KTOT] = 1.0 - 2.0 * u[:KTOT]
        ind = np.asarray(indicator[b])
        cls = ind == -1
        pad = ind == 0
        sep = (ind > 0) & (ind % 2 == 1)
        r = (~cls).astype(np.float32)
        p = (~pad).astype(np.float32)
        q = sep.astype(np.float32)
        ones = np.ones(L, np.float32)
        # rows: [rhs over keys | lhsT over this core's queries]
        m2_rhs = np.stack([r, r * q])  # [2, L]
        m2_lhs = np.stack([101.0 * r, -100.0 * r * q])  # [2, L] -> slice
        # B3 shipped scaled by 1/100 so every entry is bf16-exact; the
        # epilogue multiplies the bank by 100.
        m3_rhs = np.stack([ones, q, p, p * q])  # [4, L]
        m3_lhs = np.stack([101.0 * ones, -100.0 * q, -101.0 * p, 100.0 * p * q])
        per_batch[b] = (
            u.astype(fp8),
            w.astype(fp8),
            m2_rhs.astype(bf16),
            m2_lhs.astype(bf16),
            m3_rhs.astype(bf16),
            m3_lhs.astype(bf16),
        )

    for c in range(N_CORES):
        b, qi = c // CORES_PER_BATCH, c % CORES_PER_BATCH
        u, w, m2_rhs, m2_lhs, m3_rhs, m3_lhs = per_batch[b]
        k = qi * QROWS
        rot = lambda a: np.ascontiguousarray(np.roll(a, -k, axis=-1))
        m2r, m2l = rot(m2_rhs), rot(m2_lhs)
        m3r, m3l = rot(m3_rhs), rot(m3_lhs)
        in_maps.append(
            {
                "u": rot(u),
                "w": rot(w),
                "m": _pack_m(
                    np.concatenate([m2r, m2l[:, :QROWS]], axis=1),
                    np.concatenate([m3r, m3l[:, :QROWS]], axis=1),
                ),
            }
        )
    return in_maps


def run(entire_top, entire_left, indicator, trace=False):
    from concourse import bass_utils

    nc = _get_nc()
    in_maps = _make_in_maps(entire_top, entire_left, indicator)
    res = bass_utils.run_bass_kernel_spmd(
        nc, in_maps, core_ids=list(range(N_CORES)), trace=trace
    )
    out = np.empty((B, L, L), np.float32)
    for c in range(N_CORES):
        b, qi = c // CORES_PER_BATCH, c % CORES_PER_BATCH
        # columns were key-rotated by -k on the way in; rotate back
        out[b, qi * QROWS : (qi + 1) * QROWS, :] = np.roll(
            res.results[c]["out"], qi * QROWS, axis=-1
        )
    return out, res


def kernel(entire_top, entire_left, indicator):
    out, _ = run(
        np.asarray(entire_top, dtype=np.float32),
        np.asarray(entire_left, dtype=np.float32),
        np.asarray(indicator),
    )
    return out
